# revision 10
# baseline (speedup 1.0000x reference)
"""Trainium2 Bass kernel for nn_ModalCoTReasoning_88536455840319.

Full-input contract: kernel(**inputs) takes the unsharded setup_inputs()
tensors and returns the full (8, 1024, 768) output.

Strategy
--------
Data-parallel over batch: core b processes batch element b (B=8, 8 cores,
no collectives needed).

Math simplifications (validated against the jax reference):
- acc is a constant matrix (all entries equal, nonzero). Then:
  * kripke mixing collapses to one 768x768 matmul + bias (folded on host)
  * the possibility-attention additive mask is a constant => softmax
    unchanged; the necessity -inf mask never fires.
- softmax computed as exp(s)/sum(exp(s)) without max subtraction
  (scores stay below ~70, far from fp32 overflow).
- the possibility attention's output projection pWp folds into the
  necessity attention's Q/K/V weights on the host.
- early-exit (vscore.mean() > 0.9) never triggers for this data; the
  device kernel exports per-step sums of `ver` so the host can detect a
  trigger and fall back to an exact host reference implementation.

v4 performance design (on top of the v2 bf16 pipeline):
- all nine per-step 768x768 projections (pq/pk/pv/nq/nk/nv/np/v1/v2) run
  in fp8e4m3 with perf_mode=DoubleRow: weights and activations are kept
  in a c-pair layout ([128 part, 2, .] tiles holding c = 256j+128g+p),
  which packs a 256-deep contraction into each matmul at 0.5 cyc/row -
  half the PE time of the bf16 version. All fp8 tensors carry power-of-2
  scale factors chosen with ~4-16x range headroom (e4m3 max 448);
  dequantization folds for free into ACT scale immediates (exp/tanh/relu),
  the VO ones column, or existing tensor_scalar constants.
- scores and AV stay bf16 (attention probabilities span too many orders
  of magnitude for fp8; host emulation: fp8 projections keep rel err at
  ~5e-3 vs the 2e-2 gate).
- fp8 activation shadows replace the bf16 ones (cur8 instead of curB,
  fp8 AOT/nec/h1), halving that SBUF footprint and DVE copy traffic.
- qk PSUM->SBUF copies batched to FD=1024 (one per head per q/k).
- all ten weight sets live in SBUF permanently; pre-ramp spin unchanged;
  emission order software-pipelines each step as in v2.
"""

import os
import math
import numpy as np

B, T, C = 8, 1024, 768
H, D = 8, 96
NSTEPS = 5
P = 128
CT = C // P            # 6 c-tiles
NPAIR = CT // 2        # 3 c-pair tiles (fp8 DoubleRow layout)
TT = T // P            # 8 t-tiles
NQ = 2                 # q chunks
QC = T // NQ           # 512
N_CORES = 8
DP = D + 1             # head stride in VO (ones column appended)
PRE_RAMP_MM = 70       # dummy matmuls to exit the PE low p-state

FP8_NAMES = ["pqT", "pkT", "pvT", "nqT", "nkT", "nvT", "npT", "v1T", "v2T"]

# power-of-2 scale factors (host emulation absmax * scale kept ~4-16x
# below the e4m3 max of 448): cur<=33, poss-AOT<=18, nec-AOT<=2.1,
# nec<=1.0, h1<=0.6; weights absmax ~0.02..0.11
S_CUR = 4.0
S_AOTP = 8.0
S_AOTN = 64.0
S_NEC = 128.0
S_H1 = 256.0

_BUILD_CACHE = {}


def _pow2_wscale(absmax):
    return float(2.0 ** math.floor(math.log2(224.0 / absmax)))


def _build(repeat=0):
    """Build the Bass program (same SPMD program for all 8 cores)."""
    from contextlib import ExitStack
    import concourse.mybir as mybir
    import concourse.tile as tile
    from concourse import bacc, library_config
    from concourse.alu_op_type import AluOpType

    F32 = mybir.dt.float32
    BF = mybir.dt.bfloat16
    F8 = mybir.dt.float8e4
    DR = mybir.MatmulPerfMode.DoubleRow
    AF = mybir.ActivationFunctionType

    nc = bacc.Bacc("TRN2", target_bir_lowering=False, debug=False,
                   num_devices=N_CORES)

    xT = nc.dram_tensor("xT", [C, T], BF, kind="ExternalInput").ap()
    m4d = nc.dram_tensor("m4T", [C, C], BF, kind="ExternalInput").ap()
    w8d = {n: nc.dram_tensor(n, [NPAIR * P, 2 * C], F8,
                             kind="ExternalInput").ap()
           for n in FP8_NAMES}
    b0d = nc.dram_tensor("b0r", [P, CT], F32, kind="ExternalInput").ap()
    b1d = nc.dram_tensor("b1r", [P, CT], F32, kind="ExternalInput").ap()
    b2d = nc.dram_tensor("b2r", [P, CT], F32, kind="ExternalInput").ap()
    outT = nc.dram_tensor("outT", [C, T], F32, kind="ExternalOutput").ap()
    vstats = nc.dram_tensor("vstats", [P, NSTEPS * CT * NQ], F32,
                            kind="ExternalOutput").ap()

    # host-chosen constants baked into the program via _prep_host symmetry
    swq = {n: _pow2_wscale(_W_ABSMAX[n]) for n in FP8_NAMES}
    exp_sc_poss = 1.0 / (swq["pqT"] * swq["pkT"] * S_CUR * S_CUR)
    exp_sc_nec = 1.0 / (swq["nqT"] * swq["nkT"] * S_AOTP * S_AOTP)
    ones_poss = swq["pvT"] * S_CUR / S_AOTP
    ones_nec = swq["nvT"] * S_AOTP / S_AOTN
    nec_sc = S_NEC / (S_AOTN * swq["npT"])
    h1_sc = S_H1 / (S_NEC * swq["v1T"])
    th_sc = 0.5 / (S_H1 * swq["v2T"])
    vv_mul = 0.5 / S_NEC

    with tile.TileContext(nc) as tc:
        with ExitStack() as ctx:
            nc.gpsimd.load_library(library_config.attn)

            persist = ctx.enter_context(tc.tile_pool(name="persist", bufs=1))
            big = ctx.enter_context(tc.tile_pool(name="big", bufs=10))
            qkp = ctx.enter_context(tc.tile_pool(name="qkp", bufs=4))
            etp = ctx.enter_context(tc.tile_pool(name="etp", bufs=12))
            vvp = ctx.enter_context(tc.tile_pool(name="vvp", bufs=2))
            pvs = ctx.enter_context(tc.tile_pool(name="pvs", bufs=3))
            rp = ctx.enter_context(tc.tile_pool(name="rp", bufs=2))
            rbp = ctx.enter_context(tc.tile_pool(name="rbp", bufs=3))
            pj = ctx.enter_context(tc.tile_pool(name="pj", bufs=2, space="PSUM"))
            psc = ctx.enter_context(tc.tile_pool(name="psc", bufs=2, space="PSUM"))
            pv = ctx.enter_context(tc.tile_pool(name="pv", bufs=2, space="PSUM"))

            # ---------------- persistent state ----------------
            m4w = [persist.tile([P, C], BF, tag=f"w_m4_{ct}",
                                name=f"w_m4_{ct}") for ct in range(CT)]
            w8 = {n: [persist.tile([P, 2, C], F8, tag=f"w8_{n}_{j}",
                                   name=f"w8_{n}_{j}") for j in range(NPAIR)]
                  for n in FP8_NAMES}
            curT = [persist.tile([P, T], F32, tag=f"cur{i}", name=f"curT{i}")
                    for i in range(CT)]
            cur8 = [persist.tile([P, 2, T], F8, tag=f"cur8_{j}",
                                 name=f"cur8_{j}") for j in range(NPAIR)]
            # two VO sets: the softmax-denominator ones columns carry the
            # per-attention dequant constant, written once at init
            VOp = [persist.tile([P, H * DP], BF, tag=f"vop{i}",
                                name=f"VOp{i}") for i in range(TT)]
            VOn = [persist.tile([P, H * DP], BF, tag=f"von{i}",
                                name=f"VOn{i}") for i in range(TT)]
            b0s = persist.tile([P, CT], F32, tag="b0s")
            b1s = persist.tile([P, CT], F32, tag="b1s")
            b2s = persist.tile([P, CT], F32, tag="b2s")
            vst = persist.tile([P, NSTEPS * CT * NQ], F32, tag="vst")
            dummy = persist.tile([P, QC], BF, tag="dummy")
            onesp = persist.tile([P, 1], F32, tag="onesp")
            onesn = persist.tile([P, 1], F32, tag="onesn")

            # pre-ramp spin: PE exits the 1.2GHz p-state (~30us) while the
            # input DMAs land
            nc.vector.memset(dummy, 0.0)
            nc.vector.memset(vst, 0.0)
            for i in range(PRE_RAMP_MM // 2):
                pd = pj.tile([P, T], F32, tag="pj", name=f"preramp{i}")
                nc.tensor.matmul(pd[:, :QC], lhsT=dummy[:, :P], rhs=dummy,
                                 start=True, stop=True)
                nc.tensor.matmul(pd[:, QC:], lhsT=dummy[:, :P], rhs=dummy,
                                 start=True, stop=True)

            # one-time setup: weight/bias DMAs + VO ones columns
            for ct in range(CT):
                nc.sync.dma_start(out=m4w[ct],
                                  in_=m4d[ct * P:(ct + 1) * P, :])
            nc.sync.dma_start(out=b0s, in_=b0d)
            nc.sync.dma_start(out=b1s, in_=b1d)
            nc.sync.dma_start(out=b2s, in_=b2d)
            for n in FP8_NAMES:
                for j in range(NPAIR):
                    nc.sync.dma_start(out=w8[n][j],
                                      in_=w8d[n][j * P:(j + 1) * P, :])

            nc.vector.memset(onesp, ones_poss)
            nc.vector.memset(onesn, ones_nec)
            for i in range(TT):
                for h in range(H):
                    nc.vector.tensor_copy(
                        VOp[i][:, h * DP + D: h * DP + D + 1], onesp)
                    nc.vector.tensor_copy(
                        VOn[i][:, h * DP + D: h * DP + D + 1], onesn)

            if repeat > 0:
                ctx.enter_context(tc.For_i(0, repeat, 1))

            xs = [big.tile([P, T], BF, tag="big", name=f"xs{i}")
                  for i in range(CT)]
            for ct in range(CT):
                nc.sync.dma_start(out=xs[ct], in_=xT[ct * P:(ct + 1) * P, :])

            # ---------------- kripke init (bf16): cur^T = Meff @ x^T + b0 --
            for ot in range(CT):
                pst = pj.tile([P, T], F32, tag="pj", name="pjk")
                for q in range(NQ):
                    for k in range(CT):
                        nc.tensor.matmul(
                            pst[:, q * QC:(q + 1) * QC],
                            lhsT=m4w[k][:, ot * P:(ot + 1) * P],
                            rhs=xs[k][:, q * QC:(q + 1) * QC],
                            start=(k == 0), stop=(k == CT - 1))
                for q in range(NQ):
                    qsl = slice(q * QC, (q + 1) * QC)
                    nc.scalar.activation(out=curT[ot][:, qsl], in_=pst[:, qsl],
                                         func=AF.Identity,
                                         bias=b0s[:, ot:ot + 1])
                nc.vector.tensor_scalar(
                    out=cur8[ot // 2][:, ot % 2, :], in0=curT[ot],
                    scalar1=S_CUR, scalar2=None, op0=AluOpType.mult)

            def project8(w_name, src8, epilogue):
                # fp8 DoubleRow projection: 3 pair-matmuls per (out-tile,
                # q-chunk), 256-deep contraction each
                for ot in range(CT):
                    pst = pj.tile([P, T], F32, tag="pj", name="pjt")
                    for q in range(NQ):
                        qsl = slice(q * QC, (q + 1) * QC)
                        for j in range(NPAIR):
                            nc.tensor.matmul(
                                pst[:, qsl],
                                lhsT=w8[w_name][j][:, :, ot * P:(ot + 1) * P],
                                rhs=src8[j][:, :, qsl],
                                perf_mode=DR,
                                start=(j == 0), stop=(j == NPAIR - 1))
                    epilogue(ot, pst)

            def qk_group(w_name, src8, dst, h):
                col = h * D
                pst = pj.tile([P, T], F32, tag="pj", name="pjqk")
                for q in range(NQ):
                    qsl = slice(q * QC, (q + 1) * QC)
                    for j in range(NPAIR):
                        nc.tensor.matmul(
                            pst[:D, qsl],
                            lhsT=w8[w_name][j][:, :, col:col + D],
                            rhs=src8[j][:, :, qsl],
                            perf_mode=DR,
                            start=(j == 0), stop=(j == NPAIR - 1))
                nc.vector.tensor_copy(dst, pst[:D, :])

            def attention(src8, wq_name, wk_name, wv_name, VO, exp_scale,
                          out8):
                # ---- V: [t, c] layout, scattered into VO head slots ----
                for tt in range(TT):
                    pst = pj.tile([P, T], F32, tag="pj", name="pjv")
                    for (c0, c1) in ((0, QC), (QC, C)):
                        for j in range(NPAIR):
                            nc.tensor.matmul(
                                pst[:, c0:c1],
                                lhsT=src8[j][:, :, tt * P:(tt + 1) * P],
                                rhs=w8[wv_name][j][:, :, c0:c1],
                                perf_mode=DR,
                                start=(j == 0), stop=(j == NPAIR - 1))
                    for h in range(H):
                        # split the copy at the PSUM bank edge (col 512)
                        c0, c1 = h * D, (h + 1) * D
                        cuts = [c0, QC, c1] if c0 < QC < c1 else [c0, c1]
                        for a, bb in zip(cuts, cuts[1:]):
                            nc.vector.tensor_copy(
                                VO[tt][:, h * DP + (a - c0):
                                       h * DP + (bb - c0)],
                                pst[:, a:bb])

                def emit_qk(h):
                    QTh = qkp.tile([D, T], BF, tag="qk", name=f"QT{h}")
                    KTh = qkp.tile([D, T], BF, tag="qk", name=f"KT{h}")
                    qk_group(wq_name, src8, QTh, h)
                    qk_group(wk_name, src8, KTh, h)
                    return QTh, KTh

                qk = {0: emit_qk(0), 1: emit_qk(1)}

                def scores(h, q):
                    QTh, KTh = qk[h]
                    ets = []
                    for kt in range(TT):
                        pss = psc.tile([P, QC], F32, tag="psc", name="psct")
                        nc.tensor.matmul(
                            pss,
                            lhsT=KTh[:, kt * P:(kt + 1) * P],
                            rhs=QTh[:, q * QC:(q + 1) * QC],
                            start=True, stop=True)
                        et = etp.tile([P, QC], BF, tag="et", name="et")
                        nc.scalar.activation(out=et, in_=pss, func=AF.Exp,
                                             scale=exp_scale)
                        ets.append(et)
                    return ets

                def av_mm(h, ets):
                    pav = pv.tile([DP, QC], F32, tag="pav", name="pavt")
                    for kt in range(TT):
                        nc.tensor.matmul(
                            pav,
                            lhsT=VO[kt][:, h * DP:(h + 1) * DP],
                            rhs=ets[kt],
                            start=(kt == 0), stop=(kt == TT - 1))
                    ps = pvs.tile([DP, QC], F32, tag="pvs", name="pvst")
                    nc.vector.tensor_copy(ps, pav)
                    return ps

                def av_norm(h, pss):
                    # both q chunks' reciprocals -> one [1, T] row -> a
                    # single per-head Pool broadcast
                    rt = rp.tile([1, T], BF, tag="r", name="rt")
                    with nc.allow_low_precision(
                            reason="softmax recip in bf16: 0.4% scale noise "
                                   "on attn outputs, well inside the 2e-2 "
                                   "tolerance"):
                        for q in range(NQ):
                            nc.vector.reciprocal(rt[:, q * QC:(q + 1) * QC],
                                                 pss[q][D:D + 1, :])
                    rb = rbp.tile([D, T], BF, tag="rb", name="rbt")
                    nc.gpsimd.partition_broadcast(rb, rt)
                    for q in range(NQ):
                        off = 0
                        while off < D:
                            g = D * h + off
                            if off == 0 and g % P == 0:
                                n = D
                            else:
                                n = min(32, D - off, P - (g % P))
                            ct, r0 = g // P, g % P
                            nc.vector.tensor_tensor(
                                out=out8[ct // 2][r0:r0 + n, ct % 2,
                                                  q * QC:(q + 1) * QC],
                                in0=pss[q][off:off + n, :],
                                in1=rb[off:off + n,
                                       q * QC:(q + 1) * QC],
                                op=AluOpType.mult)
                            off += n

                # software-pipelined head loop: scores(h) -> [QK(h+2)] ->
                # AV(h); the tile scheduler fills PE stalls from the next
                # ready group
                for h in range(H):
                    ets0 = scores(h, 0)
                    ets1 = scores(h, 1)
                    if h + 2 < H:
                        qk[h + 2] = emit_qk(h + 2)
                    ps0 = av_mm(h, ets0)
                    ps1 = av_mm(h, ets1)
                    av_norm(h, (ps0, ps1))

            # ---------------- 5 reasoning steps ----------------
            for step in range(NSTEPS):
                aotp = [big.tile([P, 2, T], F8, tag="big", name=f"aotp{j}")
                        for j in range(NPAIR)]
                attention(cur8, "pqT", "pkT", "pvT", VOp, exp_sc_poss, aotp)
                aotn = [big.tile([P, 2, T], F8, tag="big", name=f"aotn{j}")
                        for j in range(NPAIR)]
                attention(aotp, "nqT", "nkT", "nvT", VOn, exp_sc_nec, aotn)

                nec8 = [big.tile([P, 2, T], F8, tag="big", name=f"nec8{j}")
                        for j in range(NPAIR)]

                def ep_nec(ot, pst):
                    # nec8 = S_NEC * nec, via the ACT identity path (frees
                    # the DVE); dequant folds into the scale immediate
                    for q in range(NQ):
                        qsl = slice(q * QC, (q + 1) * QC)
                        nc.scalar.activation(
                            out=nec8[ot // 2][:, ot % 2, qsl],
                            in_=pst[:, qsl], func=AF.Identity, scale=nec_sc)
                project8("npT", aotn, ep_nec)

                h18 = [big.tile([P, 2, T], F8, tag="big", name=f"h18{j}")
                       for j in range(NPAIR)]

                def ep_h1(ot, pst):
                    # h1_8 = S_H1 * relu(v1.nec + b1): relu commutes with the
                    # positive dequant+requant scale; bias is pre-scaled on
                    # the host (b1r = S_H1 * vb1)
                    for q in range(NQ):
                        qsl = slice(q * QC, (q + 1) * QC)
                        nc.scalar.activation(
                            out=h18[ot // 2][:, ot % 2, qsl],
                            in_=pst[:, qsl], func=AF.Relu, scale=h1_sc,
                            bias=b1s[:, ot:ot + 1])
                project8("v1T", nec8, ep_h1)

                def ep_ver(ot, pst):
                    # ver = sigmoid(z+b2) = 0.5*tanh(0.5*z + 0.5*b2) + 0.5;
                    # vstats accumulates sum(tanh); the host reconstructs
                    # sum(ver) = 0.5*sum(tanh) + 0.5*n. ver*nec uses the fp8
                    # nec8 shadow with the 1/S_NEC dequant folded into the
                    # tensor_scalar constants.
                    th = vvp.tile([P, T], BF, tag="vv", name="th")
                    for q in range(NQ):
                        idx = (step * CT + ot) * NQ + q
                        qsl = slice(q * QC, (q + 1) * QC)
                        nc.scalar.activation(out=th[:, qsl], in_=pst[:, qsl],
                                             func=AF.Tanh, scale=th_sc,
                                             bias=b2s[:, ot:ot + 1],
                                             accum_out=vst[:, idx:idx + 1])
                    vv = vvp.tile([P, T], BF, tag="vv", name="vv")
                    nc.vector.tensor_scalar(out=vv, in0=th, scalar1=vv_mul,
                                            scalar2=vv_mul, op0=AluOpType.mult,
                                            op1=AluOpType.add)
                    nc.vector.tensor_tensor(out=vv, in0=vv,
                                            in1=nec8[ot // 2][:, ot % 2, :],
                                            op=AluOpType.mult)
                    nc.vector.tensor_tensor(out=curT[ot], in0=curT[ot],
                                            in1=vv, op=AluOpType.add)
                    nc.vector.tensor_scalar(
                        out=cur8[ot // 2][:, ot % 2, :], in0=curT[ot],
                        scalar1=S_CUR, scalar2=None, op0=AluOpType.mult)
                project8("v2T", h18, ep_ver)

            for ct in range(CT):
                nc.sync.dma_start(out=outT[ct * P:(ct + 1) * P, :],
                                  in_=curT[ct])
            nc.sync.dma_start(out=vstats, in_=vst)

    nc.compile()
    return nc


_W_ABSMAX = {}


def _get_build(repeat=0):
    key = (repeat, tuple(sorted(_W_ABSMAX.items())))
    if key not in _BUILD_CACHE:
        _BUILD_CACHE[key] = _build(repeat)
    return _BUILD_CACHE[key]


def _prep_host(inp, unused=None):
    """Fold/transpose/quantize weights on the host."""
    import ml_dtypes
    e4m3 = ml_dtypes.float8_e4m3fn
    bf16 = ml_dtypes.bfloat16
    f = np.float32
    x = np.asarray(inp["x"], f)
    acc = np.asarray(inp["acc"], f)
    world_emb = np.asarray(inp["world_emb"], f)
    mixer_W = np.asarray(inp["mixer_W"], f)
    mixer_b = np.asarray(inp["mixer_b"], f)
    W = acc.shape[0]

    # kripke collapse (valid for any acc):
    a = acc.sum(axis=1)
    blocks = mixer_W.reshape(C, W, C)
    Meff = np.einsum("i,oic->oc", a, blocks).astype(f)
    ci = acc @ world_emb
    bias0 = (np.einsum("ic,oic->o", ci, blocks) + mixer_b).astype(f)

    sc = f(1.0 / math.sqrt(D))
    pWp64 = np.asarray(inp["pWp"], np.float64)
    nq_f = ((np.asarray(inp["nWq"], np.float64) * float(sc)) @ pWp64).astype(f)
    nk_f = (np.asarray(inp["nWk"], np.float64) @ pWp64).astype(f)
    nv_f = (np.asarray(inp["nWv"], np.float64) @ pWp64).astype(f)
    wT = {
        "pqT": (np.asarray(inp["pWq"], f) * sc).T,
        "pkT": np.asarray(inp["pWk"], f).T,
        "pvT": np.asarray(inp["pWv"], f).T,
        "nqT": nq_f.T,
        "nkT": nk_f.T,
        "nvT": nv_f.T,
        "npT": np.asarray(inp["nWp"], f).T,
        "v1T": np.asarray(inp["vW1"], f).T,
        "v2T": np.asarray(inp["vW2"], f).T,
    }
    global _W_ABSMAX
    _W_ABSMAX = {n: float(np.abs(v).max()) for n, v in wT.items()}

    m = {}
    for n, v in wT.items():
        s = _pow2_wscale(_W_ABSMAX[n])
        # c-pair layout: rows j*128+p, cols g*768+m hold wT[256j+128g+p, m]
        vq = np.ascontiguousarray(
            (v * s).reshape(NPAIR, 2, P, C).transpose(0, 2, 1, 3)
            .reshape(NPAIR * P, 2 * C)).astype(e4m3)
        m[n] = vq
    m["m4T"] = np.ascontiguousarray(Meff.T.astype(bf16))
    m["b0r"] = np.ascontiguousarray(bias0.reshape(CT, P).T)
    m["b1r"] = np.ascontiguousarray(
        (np.asarray(inp["vb1"], f) * S_H1).reshape(CT, P).T)
    m["b2r"] = np.ascontiguousarray(
        (np.asarray(inp["vb2"], f) * 0.5).reshape(CT, P).T)
    m["scalars"] = np.zeros((1, 8), f)
    xTs = [np.ascontiguousarray(x[b].T.astype(bf16)) for b in range(B)]
    return m, xTs


def _host_reference(inp):
    """Faithful numpy replication of the jax reference (fallback path)."""
    f = np.float32
    x = np.asarray(inp["x"], f)
    world_emb = np.asarray(inp["world_emb"], f)
    acc = np.asarray(inp["acc"], f)
    mixer_W = np.asarray(inp["mixer_W"], f)
    mixer_b = np.asarray(inp["mixer_b"], f)
    W = acc.shape[0]

    ws = x[:, :, None, :] + world_emb[None, None, :, :]
    acc_states = np.einsum("ij,btjc->btic", acc, ws)
    combined = acc_states.reshape(x.shape[0], x.shape[1], -1)
    cur = (combined @ mixer_W.T + mixer_b).astype(f)

    Tc = x.shape[1]
    wmap = np.arange(Tc) % W
    modal_mask = acc[wmap][:, wmap]

    def modal_attn(t, Wq, Wk, Wv, Wp, modal_w, use_necessity):
        Bc, Tn, Cc = t.shape
        q = (t @ Wq.T).reshape(Bc, Tn, H, D).transpose(0, 2, 1, 3)
        k = (t @ Wk.T).reshape(Bc, Tn, H, D).transpose(0, 2, 1, 3)
        v = (t @ Wv.T).reshape(Bc, Tn, H, D).transpose(0, 2, 1, 3)
        scores = (q @ k.transpose(0, 1, 3, 2)) / math.sqrt(D)
        if use_necessity:
            scores = np.where(modal_mask[None, None] == 0, -np.inf, scores)
        else:
            scores = scores + modal_w * modal_mask[None, None]
        scores = scores - scores.max(axis=-1, keepdims=True)
        e = np.exp(scores)
        a = e / e.sum(axis=-1, keepdims=True)
        o = (a @ v).transpose(0, 2, 1, 3).reshape(Bc, Tn, Cc)
        return (o @ Wp.T).astype(f)

    done = False
    for _ in range(NSTEPS):
        poss = modal_attn(cur, np.asarray(inp["pWq"], f),
                          np.asarray(inp["pWk"], f), np.asarray(inp["pWv"], f),
                          np.asarray(inp["pWp"], f), f(inp["p_mw"]), False)
        nec = modal_attn(poss, np.asarray(inp["nWq"], f),
                         np.asarray(inp["nWk"], f), np.asarray(inp["nWv"], f),
                         np.asarray(inp["nWp"], f), f(inp["n_mw"]), True)
        h = np.maximum(nec @ np.asarray(inp["vW1"], f).T
                       + np.asarray(inp["vb1"], f), 0)
        ver = 1.0 / (1.0 + np.exp(-(h @ np.asarray(inp["vW2"], f).T
                                    + np.asarray(inp["vb2"], f))))
        vscore = ver.mean(axis=-1)
        if not done:
            cur = (cur + ver * nec).astype(f)
        done = done or (vscore.mean() > 0.9)
    return cur


_LAST_RESULTS = None  # test harness introspection


def kernel(**inputs):
    global _LAST_RESULTS
    x = np.asarray(inputs["x"], np.float32)
    acc = np.asarray(inputs["acc"], np.float32)

    structural_ok = (
        x.shape == (B, T, C)
        and acc.shape[0] == acc.shape[1]
        and np.all(acc == acc.flat[0])
        and acc.flat[0] != 0.0
    )
    if not structural_ok:
        return _host_reference(inputs)

    from concourse.bass_utils import run_bass_kernel_spmd

    shared, xTs = _prep_host(inputs)
    nc = _get_build()
    in_maps = [dict(shared, xT=xTs[b]) for b in range(B)]

    res = None
    for attempt in range(2):
        try:
            res = run_bass_kernel_spmd(nc, in_maps, list(range(N_CORES)))
            break
        except Exception:
            if attempt == 1:
                return _host_reference(inputs)
    _LAST_RESULTS = res

    # early-exit guard: reference stops updating cur once the *global*
    # (cross-batch) mean of ver exceeds 0.9 at the end of a step.
    vs = np.stack([r["vstats"] for r in res.results])   # (B, 128, 60)
    done = False
    for s in range(NSTEPS):
        cols = slice(s * CT * NQ, (s + 1) * CT * NQ)
        mean_s = 0.5 * vs[:, :, cols].sum() / (B * T * C) + 0.5
        if done:
            return _host_reference(inputs)
        done = done or (mean_s > 0.9)

    out = np.empty((B, T, C), np.float32)
    for b in range(B):
        out[b] = res.results[b]["outT"].T
    return out


if __name__ == "__main__":
    _d = np.load("/root/problem/inputs.npz")
    _prep_host({k: _d[k] for k in _d.files})
    nc = _get_build()
    print("build ok")


# revision 12
# speedup vs baseline: 1.2176x; 1.2176x over previous
"""Trainium2 Bass kernel for nn_ModalCoTReasoning_88536455840319.

Full-input contract: kernel(**inputs) takes the unsharded setup_inputs()
tensors and returns the full (8, 1024, 768) output.

Strategy
--------
Data-parallel over batch: core b processes batch element b (B=8, 8 cores,
no collectives needed).

Math simplifications (validated against the jax reference):
- acc is a constant matrix (all entries equal, nonzero). Then:
  * kripke mixing collapses to one 768x768 matmul + bias (folded on host)
  * the possibility-attention additive mask is a constant => softmax
    unchanged; the necessity -inf mask never fires.
- softmax computed as exp(s)/sum(exp(s)) without max subtraction
  (scores stay below ~70, far from fp32 overflow).
- the possibility attention's output projection pWp folds into the
  necessity attention's Q/K/V weights on the host.
- early-exit (vscore.mean() > 0.9) never triggers for this data; the
  device kernel exports per-step sums of `ver` so the host can detect a
  trigger and fall back to an exact host reference implementation.

v4 performance design (on top of the v2 bf16 pipeline):
- all nine per-step 768x768 projections (pq/pk/pv/nq/nk/nv/np/v1/v2) run
  in fp8e4m3 with perf_mode=DoubleRow: weights and activations are kept
  in a c-pair layout ([128 part, 2, .] tiles holding c = 256j+128g+p),
  which packs a 256-deep contraction into each matmul at 0.5 cyc/row -
  half the PE time of the bf16 version. All fp8 tensors carry power-of-2
  scale factors chosen with ~4-16x range headroom (e4m3 max 448);
  dequantization folds for free into ACT scale immediates (exp/tanh/relu),
  the VO ones column, or existing tensor_scalar constants.
- scores and AV stay bf16 (attention probabilities span too many orders
  of magnitude for fp8; host emulation: fp8 projections keep rel err at
  ~5e-3 vs the 2e-2 gate).
- fp8 activation shadows replace the bf16 ones (cur8 instead of curB,
  fp8 AOT/nec/h1), halving that SBUF footprint and DVE copy traffic.
- qk PSUM->SBUF copies batched to FD=1024 (one per head per q/k).
- all ten weight sets live in SBUF permanently; pre-ramp spin unchanged;
  emission order software-pipelines each step as in v2.
"""

import os
import math
import numpy as np

B, T, C = 8, 1024, 768
H, D = 8, 96
NSTEPS = 5
P = 128
CT = C // P            # 6 c-tiles
NPAIR = CT // 2        # 3 c-pair tiles (fp8 DoubleRow layout)
TT = T // P            # 8 t-tiles
NQ = 2                 # q chunks
QC = T // NQ           # 512
N_CORES = 8
DP = D + 1             # head stride in VO (ones column appended)
PRE_RAMP_MM = 70       # dummy matmuls to exit the PE low p-state

FP8_NAMES = ["pqT", "pkT", "pvT", "nqT", "nkT", "nvT", "npT", "v1T", "v2T"]

# power-of-2 scale factors (host emulation absmax * scale kept ~4-16x
# below the e4m3 max of 448): cur<=33, poss-AOT<=18, nec-AOT<=2.1,
# nec<=1.0, h1<=0.6; weights absmax ~0.02..0.11
S_CUR = 4.0
S_AOTP = 8.0
S_AOTN = 64.0
S_NEC = 128.0
S_H1 = 256.0

_BUILD_CACHE = {}


def _pow2_wscale(absmax):
    return float(2.0 ** math.floor(math.log2(224.0 / absmax)))


def _build(repeat=0):
    """Build the Bass program (same SPMD program for all 8 cores)."""
    from contextlib import ExitStack
    import concourse.mybir as mybir
    import concourse.tile as tile
    from concourse import bacc, library_config
    from concourse.alu_op_type import AluOpType

    F32 = mybir.dt.float32
    BF = mybir.dt.bfloat16
    F8 = mybir.dt.float8e4
    DR = mybir.MatmulPerfMode.DoubleRow
    AF = mybir.ActivationFunctionType

    nc = bacc.Bacc("TRN2", target_bir_lowering=False, debug=False,
                   num_devices=N_CORES)

    xT = nc.dram_tensor("xT", [C, T], BF, kind="ExternalInput").ap()
    m4d = nc.dram_tensor("m4T", [C, C], BF, kind="ExternalInput").ap()
    w8d = {n: nc.dram_tensor(n, [NPAIR * P, 2 * C], F8,
                             kind="ExternalInput").ap()
           for n in FP8_NAMES}
    b0d = nc.dram_tensor("b0r", [P, CT], F32, kind="ExternalInput").ap()
    b1d = nc.dram_tensor("b1r", [P, CT], F32, kind="ExternalInput").ap()
    b2d = nc.dram_tensor("b2r", [P, CT], F32, kind="ExternalInput").ap()
    outT = nc.dram_tensor("outT", [C, T], F32, kind="ExternalOutput").ap()
    vstats = nc.dram_tensor("vstats", [P, NSTEPS * CT * NQ], F32,
                            kind="ExternalOutput").ap()

    # host-chosen constants baked into the program via _prep_host symmetry
    swq = {n: _pow2_wscale(_W_ABSMAX[n]) for n in FP8_NAMES}
    exp_sc_poss = 1.0 / (swq["pqT"] * swq["pkT"] * S_CUR * S_CUR)
    exp_sc_nec = 1.0 / (swq["nqT"] * swq["nkT"] * S_AOTP * S_AOTP)
    ones_poss = swq["pvT"] * S_CUR / S_AOTP
    ones_nec = swq["nvT"] * S_AOTP / S_AOTN
    nec_sc = S_NEC / (S_AOTN * swq["npT"])
    h1_sc = S_H1 / (S_NEC * swq["v1T"])
    th_sc = 0.5 / (S_H1 * swq["v2T"])
    vv_mul = 0.5 / S_NEC

    with tile.TileContext(nc) as tc:
        with ExitStack() as ctx:
            nc.gpsimd.load_library(library_config.attn)

            persist = ctx.enter_context(tc.tile_pool(name="persist", bufs=1))
            big = ctx.enter_context(tc.tile_pool(name="big", bufs=18))
            qkp = ctx.enter_context(tc.tile_pool(name="qkp", bufs=4))
            etp = ctx.enter_context(tc.tile_pool(name="etp", bufs=12))
            vvp = ctx.enter_context(tc.tile_pool(name="vvp", bufs=2))
            pvs = ctx.enter_context(tc.tile_pool(name="pvs", bufs=3))
            rp = ctx.enter_context(tc.tile_pool(name="rp", bufs=2))
            rbp = ctx.enter_context(tc.tile_pool(name="rbp", bufs=3))
            pj = ctx.enter_context(tc.tile_pool(name="pj", bufs=2, space="PSUM"))
            psc = ctx.enter_context(tc.tile_pool(name="psc", bufs=2, space="PSUM"))
            pv = ctx.enter_context(tc.tile_pool(name="pv", bufs=2, space="PSUM"))

            # ---------------- persistent state ----------------
            m4w = [persist.tile([P, C], BF, tag=f"w_m4_{ct}",
                                name=f"w_m4_{ct}") for ct in range(CT)]
            w8 = {n: [persist.tile([P, 2, C], F8, tag=f"w8_{n}_{j}",
                                   name=f"w8_{n}_{j}") for j in range(NPAIR)]
                  for n in FP8_NAMES}
            curT = [persist.tile([P, T], F32, tag=f"cur{i}", name=f"curT{i}")
                    for i in range(CT)]
            cur8 = [persist.tile([P, 2, T], F8, tag=f"cur8_{j}",
                                 name=f"cur8_{j}") for j in range(NPAIR)]
            # two VO sets: the softmax-denominator ones columns carry the
            # per-attention dequant constant, written once at init
            VOp = [persist.tile([P, H * DP], BF, tag=f"vop{i}",
                                name=f"VOp{i}") for i in range(TT)]
            VOn = [persist.tile([P, H * DP], BF, tag=f"von{i}",
                                name=f"VOn{i}") for i in range(TT)]
            b0s = persist.tile([P, CT], F32, tag="b0s")
            b1s = persist.tile([P, CT], F32, tag="b1s")
            b2s = persist.tile([P, CT], F32, tag="b2s")
            vst = persist.tile([P, NSTEPS * CT * NQ], F32, tag="vst")
            dummy = persist.tile([P, QC], BF, tag="dummy")
            onesp = persist.tile([P, 1], F32, tag="onesp")
            onesn = persist.tile([P, 1], F32, tag="onesn")

            # pre-ramp spin: PE exits the 1.2GHz p-state (~30us) while the
            # input DMAs land
            nc.vector.memset(dummy, 0.0)
            nc.vector.memset(vst, 0.0)
            for i in range(PRE_RAMP_MM // 2):
                pd = pj.tile([P, T], F32, tag="pj", name=f"preramp{i}")
                nc.tensor.matmul(pd[:, :QC], lhsT=dummy[:, :P], rhs=dummy,
                                 start=True, stop=True)
                nc.tensor.matmul(pd[:, QC:], lhsT=dummy[:, :P], rhs=dummy,
                                 start=True, stop=True)

            # one-time setup: weight/bias DMAs + VO ones columns
            for ct in range(CT):
                nc.sync.dma_start(out=m4w[ct],
                                  in_=m4d[ct * P:(ct + 1) * P, :])
            nc.sync.dma_start(out=b0s, in_=b0d)
            nc.sync.dma_start(out=b1s, in_=b1d)
            nc.sync.dma_start(out=b2s, in_=b2d)
            for n in FP8_NAMES:
                for j in range(NPAIR):
                    nc.sync.dma_start(out=w8[n][j],
                                      in_=w8d[n][j * P:(j + 1) * P, :])

            nc.vector.memset(onesp, ones_poss)
            nc.vector.memset(onesn, ones_nec)
            for i in range(TT):
                for h in range(H):
                    nc.vector.tensor_copy(
                        VOp[i][:, h * DP + D: h * DP + D + 1], onesp)
                    nc.vector.tensor_copy(
                        VOn[i][:, h * DP + D: h * DP + D + 1], onesn)

            if repeat > 0:
                ctx.enter_context(tc.For_i(0, repeat, 1))

            xs = [big.tile([P, T], BF, tag="big", name=f"xs{i}")
                  for i in range(CT)]
            for ct in range(CT):
                nc.sync.dma_start(out=xs[ct], in_=xT[ct * P:(ct + 1) * P, :])

            # ---------------- kripke init (bf16): cur^T = Meff @ x^T + b0 --
            for ot in range(CT):
                pst = pj.tile([P, T], F32, tag="pj", name="pjk")
                for q in range(NQ):
                    for k in range(CT):
                        nc.tensor.matmul(
                            pst[:, q * QC:(q + 1) * QC],
                            lhsT=m4w[k][:, ot * P:(ot + 1) * P],
                            rhs=xs[k][:, q * QC:(q + 1) * QC],
                            start=(k == 0), stop=(k == CT - 1))
                for q in range(NQ):
                    qsl = slice(q * QC, (q + 1) * QC)
                    nc.scalar.activation(out=curT[ot][:, qsl], in_=pst[:, qsl],
                                         func=AF.Identity,
                                         bias=b0s[:, ot:ot + 1])
                nc.vector.tensor_scalar(
                    out=cur8[ot // 2][:, ot % 2, :], in0=curT[ot],
                    scalar1=S_CUR, scalar2=None, op0=AluOpType.mult)

            def project8(w_name, src8, epilogue):
                # fp8 DoubleRow projection: 3 pair-matmuls per (out-tile,
                # q-chunk), 256-deep contraction each
                for ot in range(CT):
                    pst = pj.tile([P, T], F32, tag="pj", name="pjt")
                    for q in range(NQ):
                        qsl = slice(q * QC, (q + 1) * QC)
                        for j in range(NPAIR):
                            nc.tensor.matmul(
                                pst[:, qsl],
                                lhsT=w8[w_name][j][:, :, ot * P:(ot + 1) * P],
                                rhs=src8[j][:, :, qsl],
                                perf_mode=DR,
                                start=(j == 0), stop=(j == NPAIR - 1))
                    epilogue(ot, pst)

            def qk_group(w_name, src8, dst, h):
                col = h * D
                pst = pj.tile([P, T], F32, tag="pj", name="pjqk")
                for q in range(NQ):
                    qsl = slice(q * QC, (q + 1) * QC)
                    for j in range(NPAIR):
                        nc.tensor.matmul(
                            pst[:D, qsl],
                            lhsT=w8[w_name][j][:, :, col:col + D],
                            rhs=src8[j][:, :, qsl],
                            perf_mode=DR,
                            start=(j == 0), stop=(j == NPAIR - 1))
                for q in range(NQ):
                    qsl = slice(q * QC, (q + 1) * QC)
                    nc.vector.tensor_copy(dst[:, qsl], pst[:D, qsl])

            def attention(src8, wq_name, wk_name, wv_name, VO, exp_scale,
                          out8):
                # ---- V: [t, c] layout, scattered into VO head slots ----
                for tt in range(TT):
                    pst = pj.tile([P, T], F32, tag="pj", name="pjv")
                    for (c0, c1) in ((0, QC), (QC, C)):
                        for j in range(NPAIR):
                            nc.tensor.matmul(
                                pst[:, c0:c1],
                                lhsT=src8[j][:, :, tt * P:(tt + 1) * P],
                                rhs=w8[wv_name][j][:, :, c0:c1],
                                perf_mode=DR,
                                start=(j == 0), stop=(j == NPAIR - 1))
                    for h in range(H):
                        # split the copy at the PSUM bank edge (col 512)
                        c0, c1 = h * D, (h + 1) * D
                        cuts = [c0, QC, c1] if c0 < QC < c1 else [c0, c1]
                        for a, bb in zip(cuts, cuts[1:]):
                            nc.vector.tensor_copy(
                                VO[tt][:, h * DP + (a - c0):
                                       h * DP + (bb - c0)],
                                pst[:, a:bb])

                def emit_qk(h):
                    QTh = qkp.tile([D, T], BF, tag="qk", name=f"QT{h}")
                    KTh = qkp.tile([D, T], BF, tag="qk", name=f"KT{h}")
                    qk_group(wq_name, src8, QTh, h)
                    qk_group(wk_name, src8, KTh, h)
                    return QTh, KTh

                qk = {0: emit_qk(0), 1: emit_qk(1)}

                def scores(h, q):
                    QTh, KTh = qk[h]
                    ets = []
                    for kt in range(TT):
                        pss = psc.tile([P, QC], F32, tag="psc", name="psct")
                        nc.tensor.matmul(
                            pss,
                            lhsT=KTh[:, kt * P:(kt + 1) * P],
                            rhs=QTh[:, q * QC:(q + 1) * QC],
                            start=True, stop=True)
                        et = etp.tile([P, QC], BF, tag="et", name="et")
                        nc.scalar.activation(out=et, in_=pss, func=AF.Exp,
                                             scale=exp_scale)
                        ets.append(et)
                    return ets

                def av_mm(h, ets):
                    pav = pv.tile([DP, QC], F32, tag="pav", name="pavt")
                    for kt in range(TT):
                        nc.tensor.matmul(
                            pav,
                            lhsT=VO[kt][:, h * DP:(h + 1) * DP],
                            rhs=ets[kt],
                            start=(kt == 0), stop=(kt == TT - 1))
                    ps = pvs.tile([DP, QC], F32, tag="pvs", name="pvst")
                    nc.vector.tensor_copy(ps, pav)
                    return ps

                def av_norm(h, pss):
                    # both q chunks' reciprocals -> one [1, T] row -> a
                    # single per-head Pool broadcast
                    rt = rp.tile([1, T], BF, tag="r", name="rt")
                    with nc.allow_low_precision(
                            reason="softmax recip in bf16: 0.4% scale noise "
                                   "on attn outputs, well inside the 2e-2 "
                                   "tolerance"):
                        for q in range(NQ):
                            nc.vector.reciprocal(rt[:, q * QC:(q + 1) * QC],
                                                 pss[q][D:D + 1, :])
                    rb = rbp.tile([D, T], BF, tag="rb", name="rbt")
                    nc.gpsimd.partition_broadcast(rb, rt)
                    for q in range(NQ):
                        off = 0
                        while off < D:
                            g = D * h + off
                            if off == 0 and g % P == 0:
                                n = D
                            else:
                                n = min(32, D - off, P - (g % P))
                            ct, r0 = g // P, g % P
                            nc.vector.tensor_tensor(
                                out=out8[ct // 2][r0:r0 + n, ct % 2,
                                                  q * QC:(q + 1) * QC],
                                in0=pss[q][off:off + n, :],
                                in1=rb[off:off + n,
                                       q * QC:(q + 1) * QC],
                                op=AluOpType.mult)
                            off += n

                # software-pipelined head loop: scores(h) -> [QK(h+2)] ->
                # AV(h); the tile scheduler fills PE stalls from the next
                # ready group
                for h in range(H):
                    ets0 = scores(h, 0)
                    ets1 = scores(h, 1)
                    if h + 2 < H:
                        qk[h + 2] = emit_qk(h + 2)
                    ps0 = av_mm(h, ets0)
                    ps1 = av_mm(h, ets1)
                    av_norm(h, (ps0, ps1))

            # ---------------- 5 reasoning steps ----------------
            for step in range(NSTEPS):
                aotp = [big.tile([P, 2, T], F8, tag="big", name=f"aotp{j}")
                        for j in range(NPAIR)]
                attention(cur8, "pqT", "pkT", "pvT", VOp, exp_sc_poss, aotp)
                aotn = [big.tile([P, 2, T], F8, tag="big", name=f"aotn{j}")
                        for j in range(NPAIR)]
                attention(aotp, "nqT", "nkT", "nvT", VOn, exp_sc_nec, aotn)

                nec8 = [big.tile([P, 2, T], F8, tag="big", name=f"nec8{j}")
                        for j in range(NPAIR)]

                def ep_nec(ot, pst):
                    # nec8 = S_NEC * nec, via the ACT identity path (frees
                    # the DVE); dequant folds into the scale immediate
                    for q in range(NQ):
                        qsl = slice(q * QC, (q + 1) * QC)
                        nc.scalar.activation(
                            out=nec8[ot // 2][:, ot % 2, qsl],
                            in_=pst[:, qsl], func=AF.Identity, scale=nec_sc)
                project8("npT", aotn, ep_nec)

                h18 = [big.tile([P, 2, T], F8, tag="big", name=f"h18{j}")
                       for j in range(NPAIR)]

                def ep_h1(ot, pst):
                    # h1_8 = S_H1 * relu(v1.nec + b1): relu commutes with the
                    # positive dequant+requant scale; bias is pre-scaled on
                    # the host (b1r = S_H1 * vb1)
                    for q in range(NQ):
                        qsl = slice(q * QC, (q + 1) * QC)
                        nc.scalar.activation(
                            out=h18[ot // 2][:, ot % 2, qsl],
                            in_=pst[:, qsl], func=AF.Relu, scale=h1_sc,
                            bias=b1s[:, ot:ot + 1])
                project8("v1T", nec8, ep_h1)

                def ep_ver(ot, pst):
                    # ver = sigmoid(z+b2) = 0.5*tanh(0.5*z + 0.5*b2) + 0.5;
                    # vstats accumulates sum(tanh); the host reconstructs
                    # sum(ver) = 0.5*sum(tanh) + 0.5*n. ver*nec uses the fp8
                    # nec8 shadow with the 1/S_NEC dequant folded into the
                    # tensor_scalar constants.
                    th = vvp.tile([P, T], BF, tag="vv", name="th")
                    for q in range(NQ):
                        idx = (step * CT + ot) * NQ + q
                        qsl = slice(q * QC, (q + 1) * QC)
                        nc.scalar.activation(out=th[:, qsl], in_=pst[:, qsl],
                                             func=AF.Tanh, scale=th_sc,
                                             bias=b2s[:, ot:ot + 1],
                                             accum_out=vst[:, idx:idx + 1])
                    vv = vvp.tile([P, T], BF, tag="vv", name="vv")
                    nc.vector.tensor_scalar(out=vv, in0=th, scalar1=vv_mul,
                                            scalar2=vv_mul, op0=AluOpType.mult,
                                            op1=AluOpType.add)
                    nc.vector.tensor_tensor(out=vv, in0=vv,
                                            in1=nec8[ot // 2][:, ot % 2, :],
                                            op=AluOpType.mult)
                    nc.vector.tensor_tensor(out=curT[ot], in0=curT[ot],
                                            in1=vv, op=AluOpType.add)
                    nc.vector.tensor_scalar(
                        out=cur8[ot // 2][:, ot % 2, :], in0=curT[ot],
                        scalar1=S_CUR, scalar2=None, op0=AluOpType.mult)
                project8("v2T", h18, ep_ver)

            for ct in range(CT):
                nc.sync.dma_start(out=outT[ct * P:(ct + 1) * P, :],
                                  in_=curT[ct])
            nc.sync.dma_start(out=vstats, in_=vst)

    nc.compile()
    return nc


_W_ABSMAX = {}


def _get_build(repeat=0):
    key = (repeat, tuple(sorted(_W_ABSMAX.items())))
    if key not in _BUILD_CACHE:
        _BUILD_CACHE[key] = _build(repeat)
    return _BUILD_CACHE[key]


def _prep_host(inp, unused=None):
    """Fold/transpose/quantize weights on the host."""
    import ml_dtypes
    e4m3 = ml_dtypes.float8_e4m3fn
    bf16 = ml_dtypes.bfloat16
    f = np.float32
    x = np.asarray(inp["x"], f)
    acc = np.asarray(inp["acc"], f)
    world_emb = np.asarray(inp["world_emb"], f)
    mixer_W = np.asarray(inp["mixer_W"], f)
    mixer_b = np.asarray(inp["mixer_b"], f)
    W = acc.shape[0]

    # kripke collapse (valid for any acc):
    a = acc.sum(axis=1)
    blocks = mixer_W.reshape(C, W, C)
    Meff = np.einsum("i,oic->oc", a, blocks).astype(f)
    ci = acc @ world_emb
    bias0 = (np.einsum("ic,oic->o", ci, blocks) + mixer_b).astype(f)

    sc = f(1.0 / math.sqrt(D))
    pWp64 = np.asarray(inp["pWp"], np.float64)
    nq_f = ((np.asarray(inp["nWq"], np.float64) * float(sc)) @ pWp64).astype(f)
    nk_f = (np.asarray(inp["nWk"], np.float64) @ pWp64).astype(f)
    nv_f = (np.asarray(inp["nWv"], np.float64) @ pWp64).astype(f)
    wT = {
        "pqT": (np.asarray(inp["pWq"], f) * sc).T,
        "pkT": np.asarray(inp["pWk"], f).T,
        "pvT": np.asarray(inp["pWv"], f).T,
        "nqT": nq_f.T,
        "nkT": nk_f.T,
        "nvT": nv_f.T,
        "npT": np.asarray(inp["nWp"], f).T,
        "v1T": np.asarray(inp["vW1"], f).T,
        "v2T": np.asarray(inp["vW2"], f).T,
    }
    global _W_ABSMAX
    _W_ABSMAX = {n: float(np.abs(v).max()) for n, v in wT.items()}

    m = {}
    for n, v in wT.items():
        s = _pow2_wscale(_W_ABSMAX[n])
        # c-pair layout: rows j*128+p, cols g*768+m hold wT[256j+128g+p, m]
        vq = np.ascontiguousarray(
            (v * s).reshape(NPAIR, 2, P, C).transpose(0, 2, 1, 3)
            .reshape(NPAIR * P, 2 * C)).astype(e4m3)
        m[n] = vq
    m["m4T"] = np.ascontiguousarray(Meff.T.astype(bf16))
    m["b0r"] = np.ascontiguousarray(bias0.reshape(CT, P).T)
    m["b1r"] = np.ascontiguousarray(
        (np.asarray(inp["vb1"], f) * S_H1).reshape(CT, P).T)
    m["b2r"] = np.ascontiguousarray(
        (np.asarray(inp["vb2"], f) * 0.5).reshape(CT, P).T)
    m["scalars"] = np.zeros((1, 8), f)
    xTs = [np.ascontiguousarray(x[b].T.astype(bf16)) for b in range(B)]
    return m, xTs


def _host_reference(inp):
    """Faithful numpy replication of the jax reference (fallback path)."""
    f = np.float32
    x = np.asarray(inp["x"], f)
    world_emb = np.asarray(inp["world_emb"], f)
    acc = np.asarray(inp["acc"], f)
    mixer_W = np.asarray(inp["mixer_W"], f)
    mixer_b = np.asarray(inp["mixer_b"], f)
    W = acc.shape[0]

    ws = x[:, :, None, :] + world_emb[None, None, :, :]
    acc_states = np.einsum("ij,btjc->btic", acc, ws)
    combined = acc_states.reshape(x.shape[0], x.shape[1], -1)
    cur = (combined @ mixer_W.T + mixer_b).astype(f)

    Tc = x.shape[1]
    wmap = np.arange(Tc) % W
    modal_mask = acc[wmap][:, wmap]

    def modal_attn(t, Wq, Wk, Wv, Wp, modal_w, use_necessity):
        Bc, Tn, Cc = t.shape
        q = (t @ Wq.T).reshape(Bc, Tn, H, D).transpose(0, 2, 1, 3)
        k = (t @ Wk.T).reshape(Bc, Tn, H, D).transpose(0, 2, 1, 3)
        v = (t @ Wv.T).reshape(Bc, Tn, H, D).transpose(0, 2, 1, 3)
        scores = (q @ k.transpose(0, 1, 3, 2)) / math.sqrt(D)
        if use_necessity:
            scores = np.where(modal_mask[None, None] == 0, -np.inf, scores)
        else:
            scores = scores + modal_w * modal_mask[None, None]
        scores = scores - scores.max(axis=-1, keepdims=True)
        e = np.exp(scores)
        a = e / e.sum(axis=-1, keepdims=True)
        o = (a @ v).transpose(0, 2, 1, 3).reshape(Bc, Tn, Cc)
        return (o @ Wp.T).astype(f)

    done = False
    for _ in range(NSTEPS):
        poss = modal_attn(cur, np.asarray(inp["pWq"], f),
                          np.asarray(inp["pWk"], f), np.asarray(inp["pWv"], f),
                          np.asarray(inp["pWp"], f), f(inp["p_mw"]), False)
        nec = modal_attn(poss, np.asarray(inp["nWq"], f),
                         np.asarray(inp["nWk"], f), np.asarray(inp["nWv"], f),
                         np.asarray(inp["nWp"], f), f(inp["n_mw"]), True)
        h = np.maximum(nec @ np.asarray(inp["vW1"], f).T
                       + np.asarray(inp["vb1"], f), 0)
        ver = 1.0 / (1.0 + np.exp(-(h @ np.asarray(inp["vW2"], f).T
                                    + np.asarray(inp["vb2"], f))))
        vscore = ver.mean(axis=-1)
        if not done:
            cur = (cur + ver * nec).astype(f)
        done = done or (vscore.mean() > 0.9)
    return cur


_LAST_RESULTS = None  # test harness introspection


def kernel(**inputs):
    global _LAST_RESULTS
    x = np.asarray(inputs["x"], np.float32)
    acc = np.asarray(inputs["acc"], np.float32)

    structural_ok = (
        x.shape == (B, T, C)
        and acc.shape[0] == acc.shape[1]
        and np.all(acc == acc.flat[0])
        and acc.flat[0] != 0.0
    )
    if not structural_ok:
        return _host_reference(inputs)

    from concourse.bass_utils import run_bass_kernel_spmd

    shared, xTs = _prep_host(inputs)
    nc = _get_build()
    in_maps = [dict(shared, xT=xTs[b]) for b in range(B)]

    res = None
    for attempt in range(2):
        try:
            res = run_bass_kernel_spmd(nc, in_maps, list(range(N_CORES)))
            break
        except Exception:
            if attempt == 1:
                return _host_reference(inputs)
    _LAST_RESULTS = res

    # early-exit guard: reference stops updating cur once the *global*
    # (cross-batch) mean of ver exceeds 0.9 at the end of a step.
    vs = np.stack([r["vstats"] for r in res.results])   # (B, 128, 60)
    done = False
    for s in range(NSTEPS):
        cols = slice(s * CT * NQ, (s + 1) * CT * NQ)
        mean_s = 0.5 * vs[:, :, cols].sum() / (B * T * C) + 0.5
        if done:
            return _host_reference(inputs)
        done = done or (mean_s > 0.9)

    out = np.empty((B, T, C), np.float32)
    for b in range(B):
        out[b] = res.results[b]["outT"].T
    return out


if __name__ == "__main__":
    _d = np.load("/root/problem/inputs.npz")
    _prep_host({k: _d[k] for k in _d.files})
    nc = _get_build()
    print("build ok")
